# revision 1
# baseline (speedup 1.0000x reference)
"""Masked attention kernel for Trainium2, 8 NeuronCores.

Problem: out[b,h,s,d] = softmax_t((Q@K^T masked_fill(-1e9))/sqrt(64)) @ V
  B=4, H=16, S=2048, D=64, mask [B,1,S,S] bool (True = masked).

Sharding: 64 (b,h) attention problems over 8 cores; core c handles batch c//2,
heads (c%2)*8 .. +8, so each core needs only one batch's mask.

Per-core algorithm (everything transposed: scores^T[t,s] so softmax's reduce
axis lands on the PE's contraction axis, never on DVE partitions):
  - scoresT = K^T.T @ Q^T via fp32r matmuls; contraction d=64 only half-fills
    the PE, so the two 1024-wide halves of s are row-packed via tile_position
    (0,0)/(64,0) (K^T duplicated into both partition halves) and run
    concurrently on separate row groups.
  - full-width scores tile [128, 2048] (4 PSUM banks) per t-block: softmax
    without max-subtraction (|scores| <= ~50 so exp never overflows), one
    ScalarE Exp instruction per t-block, scale=1/8.
  - mask: probs *= keepT (bf16 0/1, exact) - masked probs become exactly 0,
    matching the reference's exp(-1e9/8 - max) == 0.
  - out^T = [V | ones].T @ probsT accumulated over t in PSUM [65, 2048]
    (the other 4 banks): row 64 of the accumulator is the softmax denominator
    for free.
  - divide: DVE reciprocal of row 64, DRAM-bounce partition-broadcast, DVE
    multiply.
Host: transposes Q/K, appends ones to V, transposes+inverts mask to bf16, and
transposes the [d,s] device output back to [s,d].
"""

import numpy as np
import ml_dtypes

import concourse.tile as tile
from concourse import bacc, mybir
from concourse.bass_utils import run_bass_kernel_spmd

B, H, S, D = 4, 16, 2048, 64
N_CORES = 8
HPC = (B * H) // N_CORES  # heads per core

_NC_CACHE = {}


def build_nc(hpc=HPC, n_tb=S // 128, sq=S, niter=1):
    """Build the SPMD Bass kernel. st = key length, sq = query length."""
    st = n_tb * 128
    hw = 512          # matmul free-dim (fp32 PSUM bank)
    hq = sq // 2
    assert sq % 1024 == 0
    f32, f32r, bf16 = mybir.dt.float32, mybir.dt.float32r, mybir.dt.bfloat16
    EXP = mybir.ActivationFunctionType.Exp

    nc = bacc.Bacc("TRN2", target_bir_lowering=False, debug=False,
                   num_devices=N_CORES)
    QT = nc.dram_tensor("QT", [hpc, D, sq], f32r, kind="ExternalInput")
    KT = nc.dram_tensor("KT", [hpc, D, st], f32r, kind="ExternalInput")
    VE = nc.dram_tensor("VE", [hpc, n_tb, 128, D + 1], f32r, kind="ExternalInput")
    KP = nc.dram_tensor("KP", [n_tb, 128, sq], bf16, kind="ExternalInput")
    OT = nc.dram_tensor("OT", [hpc, D, sq], f32, kind="ExternalOutput")

    with tile.TileContext(nc) as tc:
        with (
            tc.tile_pool(name="mask", bufs=1) as maskp,
            tc.tile_pool(name="kt", bufs=2) as ktp,
            tc.tile_pool(name="ve", bufs=2) as vep,
            tc.tile_pool(name="qt", bufs=2) as qtp,
            tc.tile_pool(name="pr", bufs=2) as prp,
            tc.tile_pool(name="prm", bufs=2) as prmp,
            tc.tile_pool(name="fin", bufs=2) as finp,
            tc.tile_pool(name="dscr", bufs=2, space="DRAM") as dscrp,
            tc.tile_pool(name="sc", bufs=1, space="PSUM") as scp,
            tc.tile_pool(name="oacc", bufs=1, space="PSUM") as oaccp,
        ):
            kp = maskp.tile([128, n_tb, sq], bf16)
            nc.sync.dma_start(out=kp[:], in_=KP.ap().rearrange("n p s -> p n s"))
            for _ in range(niter):
                for h in range(hpc):
                    kt = ktp.tile([64, st], f32r)
                    nc.sync.dma_start(out=kt[:], in_=KT.ap()[h])
                    ve = vep.tile([128, n_tb, D + 1], f32r)
                    nc.sync.dma_start(out=ve[:],
                                      in_=VE.ap()[h].rearrange("n p c -> p n c"))
                    qt = qtp.tile([64, sq], f32r)
                    nc.sync.dma_start(out=qt[:], in_=QT.ap()[h])
                    oacc = oaccp.tile([D + 1, sq], f32)
                    for tb in range(n_tb):
                        t0 = tb * 128
                        sc = scp.tile([128, sq], f32)
                        for w in range(sq // hw):
                            nc.tensor.matmul(sc[:, w * hw:(w + 1) * hw],
                                             kt[:, t0:t0 + 128],
                                             qt[:, w * hw:(w + 1) * hw],
                                             start=True, stop=True)
                        pr = prp.tile([128, sq], f32)
                        nc.scalar.activation(out=pr[:], in_=sc[:], func=EXP,
                                             scale=0.125)
                        prm = prmp.tile([128, sq], f32r)
                        nc.vector.tensor_mul(prm[:], pr[:], kp[:, tb, :])
                        for w in range(sq // hw):
                            nc.tensor.matmul(oacc[:, w * hw:(w + 1) * hw],
                                             ve[:, tb, :],
                                             prm[:, w * hw:(w + 1) * hw],
                                             start=(tb == 0),
                                             stop=(tb == n_tb - 1))
                    recip = finp.tile([1, sq], f32)
                    nc.vector.reciprocal(out=recip[:], in_=oacc[D:D + 1, :])
                    dr = dscrp.tile([1, sq], f32)
                    nc.sync.dma_start(out=dr[:], in_=recip[:])
                    rb = finp.tile([D, sq], f32)
                    nc.sync.dma_start(out=rb[:], in_=dr[:].to_broadcast([D, sq]))
                    of = finp.tile([D, sq], f32)
                    nc.vector.tensor_mul(of[:], oacc[0:D, :], rb[:])
                    nc.sync.dma_start(out=OT.ap()[h], in_=of[:])
    nc.compile()
    return nc


def _get_nc(**kw):
    key = tuple(sorted(kw.items()))
    if key not in _NC_CACHE:
        _NC_CACHE[key] = build_nc(**kw)
    return _NC_CACHE[key]


def make_in_maps(Q, K, V, mask):
    """Shard full inputs into the 8 per-core input dicts."""
    bf16 = ml_dtypes.bfloat16
    QTf = np.ascontiguousarray(Q.transpose(0, 1, 3, 2), dtype=np.float32)
    KTf = np.ascontiguousarray(K.transpose(0, 1, 3, 2), dtype=np.float32)
    ones = np.ones((B, H, S, 1), np.float32)
    VEf = np.concatenate([np.asarray(V, np.float32), ones], axis=-1)
    VEf = np.ascontiguousarray(VEf).reshape(B, H, S // 128, 128, D + 1)
    # KP[b, tb, p, s] = !mask[b, 0, s, tb*128+p]
    KPf = np.ascontiguousarray(
        (~np.asarray(mask[:, 0])).transpose(0, 2, 1)).astype(bf16)
    KPf = KPf.reshape(B, S // 128, 128, S)
    in_maps = []
    for c in range(N_CORES):
        b, h0 = c // 2, (c % 2) * HPC
        in_maps.append({
            "QT": np.ascontiguousarray(QTf[b, h0:h0 + HPC]),
            "KT": np.ascontiguousarray(KTf[b, h0:h0 + HPC]),
            "VE": np.ascontiguousarray(VEf[b, h0:h0 + HPC]),
            "KP": KPf[b],
        })
    return in_maps


def kernel(Q, K, V, mask):
    nc = _get_nc()
    in_maps = make_in_maps(Q, K, V, mask)
    res = run_bass_kernel_spmd(nc, in_maps, core_ids=list(range(N_CORES)))
    out = np.empty((B, H, S, D), np.float32)
    for c in range(N_CORES):
        b, h0 = c // 2, (c % 2) * HPC
        out[b, h0:h0 + HPC] = res.results[c]["OT"].transpose(0, 2, 1)
    return out



# revision 10
# speedup vs baseline: 235.2435x; 235.2435x over previous
"""Masked attention kernel for Trainium2, 8 NeuronCores.

Problem: out[b,h,s,d] = softmax_t((Q@K^T masked_fill(-1e9))/sqrt(64)) @ V
  B=4, H=16, S=2048, D=64, mask [B,1,S,S] bool (True = masked).

Sharding: 64 (b,h) attention problems over 8 cores; core c handles batch c//2,
heads (c%2)*8 .. +8, so each core needs only one batch's mask.

Per-core algorithm (scores transposed: scoresT[t,s] so softmax's reduce axis
lands on the PE contraction axis):
  - All matmuls in bf16 (1 cyc/row on the PE vs 2+ for fp32r); rel-err budget
    is 2e-2 and bf16 end-to-end lands ~1e-3.
  - Per t-block (128 keys) x 512-query chunk: scoresT chunk = K^T.T @ Q^T into
    one PSUM bank; ScalarE Exp (scale=1/8, no max-subtraction: |scores|<~50 so
    exp never overflows fp32) -> bf16; DVE multiply by keepT (bf16 0/1, exact:
    masked probs become exactly 0, matching the reference's exp(-1e9/8)==0);
    PV accumulates [V | ones].T @ probsT into PSUM oacc [65, 2048] over the 16
    t-blocks - row 64 is the softmax denominator for free.
  - Finalize per 128-query chunk: Pool copies oacc chunk PSUM->SBUF, PE
    transposes it to [128q, 65], Pool copies back to SBUF; then one DVE
    reciprocal of the [128,16] denominator column and per-chunk DVE
    tensor_scalar multiplies (per-partition scalar broadcast) normalize.
    Output is written [q-chunk-major] so every DMA chunk is >=2KB contiguous.
Host: bf16-casts + transposes Q/K, appends ones column to V and pre-tiles it
t-block-major, inverts+transposes mask to bf16 keep, and untangles the
[128,16,64]-per-head device output back to [s,d].
"""

import numpy as np
import ml_dtypes

import concourse.tile as tile
import concourse.bass_utils as _bu
from concourse import bacc, mybir
from concourse.bass_utils import run_bass_kernel_spmd

B, H, S, D = 4, 16, 2048, 64
N_CORES = 8
HPC = (B * H) // N_CORES  # heads per core

_NC_CACHE = {}


def build_nc(hpc=HPC, n_tb=S // 128, sq=S, niter=1, elide_ldw=False):
    """Build the SPMD Bass kernel. st = key length = n_tb*128, sq = queries."""
    st = n_tb * 128
    hw = 512          # matmul free-dim chunk (one fp32 PSUM bank)
    nw = sq // hw
    f32, bf16 = mybir.dt.float32, mybir.dt.bfloat16
    EXP = mybir.ActivationFunctionType.Exp

    nc = bacc.Bacc("TRN2", target_bir_lowering=False, debug=False,
                   num_devices=N_CORES)
    QT = nc.dram_tensor("QT", [hpc, D, sq], bf16, kind="ExternalInput")
    KT = nc.dram_tensor("KT", [hpc, D, st], bf16, kind="ExternalInput")
    VE = nc.dram_tensor("VE", [hpc, 128, n_tb, D + 1], bf16,
                        kind="ExternalInput")
    KP = nc.dram_tensor("KP", [128, n_tb, sq], bf16, kind="ExternalInput")
    IDT = nc.dram_tensor("IDT", [128, 128], bf16, kind="ExternalInput")
    OT = nc.dram_tensor("OT", [hpc, 128, sq // 128, D], f32,
                        kind="ExternalOutput")

    def mm(*a, **kw):
        inst = nc.tensor.matmul(*a, **kw)
        return inst

    with tile.TileContext(nc) as tc:
        with (
            tc.tile_pool(name="mask", bufs=4) as maskp,
            tc.tile_pool(name="idt", bufs=1) as idtp,
            tc.tile_pool(name="kt", bufs=2) as ktp,
            tc.tile_pool(name="qt", bufs=2) as qtp,
            tc.tile_pool(name="ve", bufs=2) as vep,
            tc.tile_pool(name="pr", bufs=6) as prp,
            tc.tile_pool(name="prm", bufs=6) as prmp,
            tc.tile_pool(name="oc", bufs=3) as ocp,
            tc.tile_pool(name="tps", bufs=2) as tpsp,
            tc.tile_pool(name="rcp", bufs=2) as rcpp,
            tc.tile_pool(name="of", bufs=2) as ofp,
            tc.tile_pool(name="sc", bufs=2, space="PSUM") as scp,
            tc.tile_pool(name="oacc", bufs=1, space="PSUM") as oaccp,
            tc.tile_pool(name="tp", bufs=2, space="PSUM") as tpp,
        ):
            kps = []
            idt = idtp.tile([128, 128], bf16)
            tbc = n_tb // 4  # t-blocks per mask-load chunk
            first = True
            for _ in range(niter):
                for h in range(hpc):
                    kt = ktp.tile([D, st], bf16)
                    nc.sync.dma_start(out=kt[:], in_=KT.ap()[h])
                    qt = qtp.tile([D, sq], bf16)
                    nc.sync.dma_start(out=qt[:], in_=QT.ap()[h])
                    ve = vep.tile([128, n_tb, D + 1], bf16)
                    nc.sync.dma_start(out=ve[:], in_=VE.ap()[h])
                    if first:
                        # mask + identity loads queue behind the first head's
                        # tensors so the first matmuls aren't gated on 8MB.
                        for c in range(4):
                            kpc = maskp.tile([128, tbc, sq], bf16)
                            nc.sync.dma_start(
                                out=kpc[:],
                                in_=KP.ap()[:, c * tbc:(c + 1) * tbc, :])
                            kps.append(kpc)
                        nc.sync.dma_start(out=idt[:], in_=IDT.ap())
                        first = False
                    oacc = oaccp.tile([D + 1, sq], f32)
                    for tb in range(n_tb):
                        t0 = tb * 128
                        scs = []
                        for w in range(nw):
                            sc = scp.tile([128, hw], f32)
                            mm(sc[:], kt[:, t0:t0 + 128],
                               qt[:, w * hw:(w + 1) * hw],
                               start=True, stop=True)
                            scs.append(sc)
                        prms = []
                        for w in range(nw):
                            pr = prp.tile([128, hw], bf16)
                            nc.scalar.activation(out=pr[:], in_=scs[w][:],
                                                 func=EXP, scale=0.125)
                            prm = prmp.tile([128, hw], bf16)
                            # DVE is ~2.3x faster than Pool on this op;
                            # give DVE ~5/9 of the chunks.
                            eng = (nc.vector if (tb * nw + w) % 9 < 5
                                   else nc.gpsimd)
                            eng.tensor_mul(
                                prm[:], pr[:],
                                kps[tb // tbc][:, tb % tbc,
                                               w * hw:(w + 1) * hw])
                            prms.append(prm)
                        for w in range(nw):
                            mm(oacc[:, w * hw:(w + 1) * hw],
                               ve[:, tb, :], prms[w][:],
                               start=(tb == 0), stop=(tb == n_tb - 1))
                    # finalize: transpose oacc to [q, 65] chunks, normalize
                    tps = tpsp.tile([128, sq // 128, D + 1], bf16)
                    for j in range(sq // 128):
                        oc = ocp.tile([D + 1, 128], bf16)
                        nc.vector.tensor_copy(
                            out=oc[:], in_=oacc[:, j * 128:(j + 1) * 128])
                        tp = tpp.tile([128, D + 1], bf16)
                        nc.tensor.transpose(tp[:], oc[:],
                                            idt[0:D + 1, 0:D + 1])
                        nc.vector.tensor_copy(out=tps[:, j, :], in_=tp[:])
                    rcp = rcpp.tile([128, sq // 128], f32)
                    nc.vector.reciprocal(out=rcp[:], in_=tps[:, :, D])
                    of = ofp.tile([128, sq // 128, D], f32)
                    nc.vector.tensor_mul(
                        of[:], tps[:, :, 0:D],
                        rcp[:].to_broadcast([128, sq // 128, D]))
                    nc.sync.dma_start(out=OT.ap()[h], in_=of[:])
    nc.compile()
    return nc


def _get_nc(**kw):
    key = tuple(sorted(kw.items()))
    if key not in _NC_CACHE:
        _NC_CACHE[key] = build_nc(**kw)
    return _NC_CACHE[key]


def make_in_maps(Q, K, V, mask):
    """Shard full inputs into the 8 per-core input dicts."""
    bf16 = ml_dtypes.bfloat16
    QTf = np.ascontiguousarray(
        np.asarray(Q, np.float32).transpose(0, 1, 3, 2)).astype(bf16)
    KTf = np.ascontiguousarray(
        np.asarray(K, np.float32).transpose(0, 1, 3, 2)).astype(bf16)
    ones = np.ones((B, H, S, 1), np.float32)
    VEf = np.concatenate([np.asarray(V, np.float32), ones], axis=-1)
    # [B,H,S,65] -> [B,H,n_tb,128,65] -> [B,H,128,n_tb,65]
    VEf = VEf.reshape(B, H, S // 128, 128, D + 1).transpose(0, 1, 3, 2, 4)
    VEf = np.ascontiguousarray(VEf).astype(bf16)
    # KP[b, p, tb, s] = !mask[b, 0, s, tb*128+p]
    keep = (~np.asarray(mask[:, 0])).transpose(0, 2, 1)  # [B, t, s]
    keep = keep.reshape(B, S // 128, 128, S).transpose(0, 2, 1, 3)
    KPf = np.ascontiguousarray(keep).astype(bf16)
    IDTf = np.eye(128, dtype=np.float32).astype(bf16)
    in_maps = []
    for c in range(N_CORES):
        b, h0 = c // 2, (c % 2) * HPC
        in_maps.append({
            "QT": np.ascontiguousarray(QTf[b, h0:h0 + HPC]),
            "KT": np.ascontiguousarray(KTf[b, h0:h0 + HPC]),
            "VE": np.ascontiguousarray(VEf[b, h0:h0 + HPC]),
            "KP": KPf[b],
            "IDT": IDTf,
        })
    return in_maps


def kernel(Q, K, V, mask):
    nc = _get_nc()
    in_maps = make_in_maps(Q, K, V, mask)
    res = run_bass_kernel_spmd(nc, in_maps, core_ids=list(range(N_CORES)))
    out = np.empty((B, H, S, D), np.float32)
    for c in range(N_CORES):
        b, h0 = c // 2, (c % 2) * HPC
        # OT [hpc, 128, S//128, 64] -> [hpc, S, 64]
        ot = res.results[c]["OT"].transpose(0, 2, 1, 3).reshape(HPC, S, D)
        out[b, h0:h0 + HPC] = ot
    return out
